# revision 2
# baseline (speedup 1.0000x reference)
"""Trainium2 Bass kernel for nn_Embedding2Score (segment_reduce), int8 (final).

v5 = v4 with the phases FUSED per graph-block (v4 ran p1 then p2 fully
serially: 75us + 179us). Changes:
  - item_weight stays RESIDENT in SBUF (10 chunk tiles, 100KB/partition),
    so phase 2 can run block-major: right after block b's s_h is ready,
    its 50000 scores are computed while phase 1 of block b+1 proceeds.
  - s_h^T and the int8 scale are per-block tiles (shT_b, inv32_b), so
    dependencies don't serialize on a single whole-core tile.
  - Per-block inv = (127/C)/||s_h|| computed entirely on DVE with 4
    Newton sqrt iterations (seed t0 = ss/4+1 >= sqrt(ss), so t converges
    from above and the int8 quantize can never clip past 127). This
    avoids the ACT Sqrt table, which lives in a different act-table set
    than Sigmoid (1.3us reload per switch).
  - Quantize copies split ~5/12 DVE, rest ACT, per vocab tile.
"""

import sys

if "/opt/trn_rl_repo" not in sys.path:
    sys.path.insert(0, "/opt/trn_rl_repo")

import numpy as np

P = 128          # partitions / tile edge
D = 128          # hidden size
NCORES = 8
NBLK = 4         # graph blocks per core, 128 graphs each
BC = NBLK * P    # graphs per core = 512
BTOT = NCORES * BC          # 4096 graphs
V = 50000
VT = 512         # vocab tile width (one PSUM bank of f32)
VG = 10          # psum tiles per itw chunk (5120 cols, 1.25 MB f16)
NCH = 10         # itw chunks (9x5120 + 3920), all SBUF-resident
ST = 4           # 128-node subtiles per supertile
NR_ITERS = 4


def build_nc(ntpb, ckey, repeat=1, phase="both"):
    import contextlib
    import concourse.bacc as bacc
    import concourse.mybir as mybir
    from concourse.tile import TileContext

    f16 = mybir.dt.float16
    f32 = mybir.dt.float32
    i8 = mybir.dt.int8
    npb = ntpb * P
    nc = bacc.Bacc()

    xpk_ext = nc.declare_dram_parameter("xpk", [P, NBLK * npb], f16, isOutput=False)
    xtp_ext = nc.declare_dram_parameter("xtp", [P, NBLK * npb], f16, isOutput=False)
    blc_ext = nc.declare_dram_parameter("blc", [NBLK, P, ntpb], f32, isOutput=False)
    blr_ext = nc.declare_dram_parameter("blr", [NBLK, npb], f16, isOutput=False)
    vnt_ext = nc.declare_dram_parameter("vnt", [D, BC], f16, isOutput=False)
    w1t_ext = nc.declare_dram_parameter("w1t", [D, D], f16, isOutput=False)
    w2t_ext = nc.declare_dram_parameter("w2t", [D, D], f16, isOutput=False)
    w3at_ext = nc.declare_dram_parameter("w3at", [D, D], f16, isOutput=False)
    w3bt_ext = nc.declare_dram_parameter("w3bt", [D, D], f16, isOutput=False)
    b12c_ext = nc.declare_dram_parameter("b12c", [P, 1], f32, isOutput=False)
    w3bc_ext = nc.declare_dram_parameter("w3bc", [P, 1], f32, isOutput=False)
    qwt_ext = nc.declare_dram_parameter("qwt", [D, 1], f16, isOutput=False)
    qbc_ext = nc.declare_dram_parameter("qbc", [P, 1], f32, isOutput=False)
    itwt_ext = nc.declare_dram_parameter("itwt", [D, V], f16, isOutput=False)
    y_ext = nc.declare_dram_parameter("y", [BC, V], i8, isOutput=True)
    inv_ext = nc.declare_dram_parameter("invs", [P, NBLK], f16, isOutput=True)

    with TileContext(nc) as tc:
        with tc.tile_pool(name="const", bufs=1) as cp:
            iota_i = cp.tile([P, P], mybir.dt.int32, tag="iotai")
            nc.gpsimd.iota(iota_i[:], pattern=[[1, P]], base=0, channel_multiplier=0)
            iota_row = cp.tile([P, P], f16, tag="iotarow")
            nc.vector.tensor_copy(out=iota_row[:], in_=iota_i[:])
            iota_ci = cp.tile([P, 1], mybir.dt.int32, tag="iotaci")
            nc.gpsimd.iota(iota_ci[:], pattern=[[0, 1]], base=0, channel_multiplier=1)
            iota_col = cp.tile([P, 1], f32, tag="iotacol")
            nc.vector.tensor_copy(out=iota_col[:], in_=iota_ci[:])
            ones_c = cp.tile([P, 1], f16, tag="onesc")
            nc.vector.memset(ones_c[:], 1.0)

            def load(name, ext, shape, dt=f16):
                t = cp.tile(shape, dt, tag=name)
                nc.gpsimd.dma_start(out=t[:], in_=ext[:])
                return t

            w1t = load("w1t", w1t_ext, [D, D])
            w2t = load("w2t", w2t_ext, [D, D])
            w3at = load("w3at", w3at_ext, [D, D])
            w3bt = load("w3bt", w3bt_ext, [D, D])
            b12c = load("b12c", b12c_ext, [P, 1], f32)
            w3bc = load("w3bc", w3bc_ext, [P, 1], f32)
            qwt = load("qwt", qwt_ext, [D, 1])
            qbc = load("qbc", qbc_ext, [P, 1], f32)
            vnt = load("vnt", vnt_ext, [D, BC])

            rep_ctx = tc.For_i(0, repeat, 1) if repeat > 1 else contextlib.nullcontext()
            with rep_ctx:
                _build_body(nc, tc, mybir, ntpb, ckey,
                            xpk_ext, xtp_ext, blc_ext, blr_ext, itwt_ext,
                            y_ext, inv_ext,
                            iota_row, iota_col, ones_c,
                            w1t, w2t, w3at, w3bt, b12c, w3bc, qwt, qbc, vnt,
                            phase)

    nc.compile()
    return nc


def _build_body(nc, tc, mybir, ntpb, ckey,
                xpk_ext, xtp_ext, blc_ext, blr_ext, itwt_ext,
                y_ext, inv_ext,
                iota_row, iota_col, ones_c,
                w1t, w2t, w3at, w3bt, b12c, w3bc, qwt, qbc, vnt,
                phase="both"):
    f16 = mybir.dt.float16
    f32 = mybir.dt.float32
    i8 = mybir.dt.int8
    npb = ntpb * P
    nst = -(-ntpb // ST)
    Sig = mybir.ActivationFunctionType.Sigmoid
    Copy = mybir.ActivationFunctionType.Copy
    EQ = mybir.AluOpType.is_equal
    MUL = mybir.AluOpType.mult
    ADD = mybir.AluOpType.add
    p1 = phase in ("both", "p1")
    p2 = phase in ("both", "p2")
    W = VG * VT                   # 5120 cols per chunk
    csize = [min(W, V - ch * W) for ch in range(NCH)]
    # all chunk loads are emitted with block 0 (after its x loads): readers
    # in later blocks must see the writer earlier in program order
    ch_sched = {0: list(range(NCH)), 1: [], 2: [], 3: []}
    k127 = float(np.float32((ckey ** 0.5)))   # C/127

    with tc.tile_pool(name="p1big", bufs=2) as pb, \
         tc.tile_pool(name="p1", bufs=4) as pool, \
         tc.tile_pool(name="blkp", bufs=2) as blkp, \
         tc.tile_pool(name="p2i", bufs=1) as p2i, \
         tc.tile_pool(name="p2o", bufs=3) as p2o, \
         tc.tile_pool(name="gat", bufs=1) as gat, \
         tc.tile_pool(name="psPre", bufs=2, space="PSUM") as psPre, \
         tc.tile_pool(name="psBlk", bufs=1, space="PSUM") as psBlk, \
         tc.tile_pool(name="psAl", bufs=1, space="PSUM") as psAl, \
         tc.tile_pool(name="psSg", bufs=1, space="PSUM") as psSg, \
         tc.tile_pool(name="ps2", bufs=3, space="PSUM") as ps2:
        itw = []
        for ch in range(NCH):
            itw_t = p2i.tile([D, W], f16, tag=f"itw{ch}")
            itw.append(itw_t)
        invOut = gat.tile([P, NBLK], f16, tag="invout")
        qidx = 0
        for blk in range(NBLK):
            gsl = slice(blk * P, (blk + 1) * P)
            nsl = slice(blk * npb, (blk + 1) * npb)
            if p1:
                xpk = pb.tile([P, npb], f16, tag="xpk")
                nc.gpsimd.dma_start(out=xpk[:], in_=xpk_ext[:, nsl])
                xtp = pb.tile([P, npb], f16, tag="xtp")
                nc.gpsimd.dma_start(out=xtp[:], in_=xtp_ext[:, nsl])
                blc = blkp.tile([P, ntpb], f32, tag="blc")
                nc.gpsimd.dma_start(out=blc[:], in_=blc_ext[blk])
                bcb = pb.tile([P, npb], f16, tag="bcb")
                nc.gpsimd.dma_start(
                    out=bcb[:], in_=blr_ext[blk:blk + 1].to_broadcast((P, npb)))
            if p2:
                for ch in ch_sched[blk]:
                    nc.gpsimd.dma_start(out=itw[ch][:, :csize[ch]],
                                        in_=itwt_ext[:, ch * W:ch * W + csize[ch]])
            shT_b = gat.tile([D, P], f16, tag=f"shT{blk}")
            if p1:
                q1g_ps = psBlk.tile([P, P], f32, tag="blkmm", space="PSUM")
                nc.tensor.matmul(out=q1g_ps[:], lhsT=vnt[:, gsl], rhs=w1t[:],
                                 start=True, stop=True)
                q1g = blkp.tile([P, P], f16, tag="q1g")
                nc.vector.tensor_copy(out=q1g[:], in_=q1g_ps[:])

                sg_ps = psSg.tile([P, P], f32, tag="sg", space="PSUM")
                mm_i = 0
                n_mm = sum(min(ST, ntpb - ST * s) for s in range(nst))
                for st in range(nst):
                    nsub = min(ST, ntpb - ST * st)
                    w = nsub * P
                    ssl = slice(st * ST * P, st * ST * P + w)
                    StT = pool.tile([P, ST * P], f16, tag="StT")
                    nc.vector.tensor_scalar(
                        out=StT[:, :w], in0=bcb[:, ssl],
                        scalar1=iota_col[:], scalar2=None, op0=EQ)
                    pre_ps = psPre.tile([P, ST * P], f32, tag="pre", space="PSUM")
                    nc.tensor.matmul(out=pre_ps[:, :w], lhsT=w2t[:],
                                     rhs=xtp[:, ssl], start=True, stop=False)
                    nc.tensor.matmul(out=pre_ps[:, :w], lhsT=q1g[:],
                                     rhs=StT[:, :w], start=False, stop=True)
                    sigT = pool.tile([P, ST * P], f16, tag="sigT")
                    nc.scalar.activation(out=sigT[:, :w], in_=pre_ps[:, :w],
                                         func=Sig, bias=b12c[:])
                    al_ps = psAl.tile([P, ST], f32, tag="al", space="PSUM")
                    for c in range(nsub):
                        csl = slice(c * P, (c + 1) * P)
                        nc.tensor.matmul(out=al_ps[:, c:c + 1],
                                         lhsT=sigT[:, csl], rhs=qwt[:],
                                         start=True, stop=True)
                    al = pool.tile([P, ST], f32, tag="al")
                    nc.vector.tensor_scalar_add(out=al[:, :nsub],
                                                in0=al_ps[:, :nsub],
                                                scalar1=qbc[:])
                    aS = pool.tile([P, ST * P], f16, tag="aS")
                    for c in range(nsub):
                        csl = slice(c * P, (c + 1) * P)
                        nc.vector.tensor_scalar(
                            out=aS[:, csl], in0=iota_row[:],
                            scalar1=blc[:, st * ST + c:st * ST + c + 1],
                            scalar2=al[:, c:c + 1], op0=EQ, op1=MUL)
                        nc.tensor.matmul(
                            out=sg_ps[:],
                            lhsT=xpk[:, st * ST * P + c * P:st * ST * P + (c + 1) * P],
                            rhs=aS[:, csl],
                            start=(mm_i == 0), stop=(mm_i == n_mm - 1))
                        mm_i += 1

                sg_sb = blkp.tile([P, P], f16, tag="sgsb")
                nc.vector.tensor_copy(out=sg_sb[:], in_=sg_ps[:])
                sh_ps = psBlk.tile([P, P], f32, tag="blkmm", space="PSUM")
                nc.tensor.matmul(out=sh_ps[:], lhsT=w3at[:], rhs=vnt[:, gsl],
                                 start=True, stop=False)
                nc.tensor.matmul(out=sh_ps[:], lhsT=w3bt[:], rhs=sg_sb[:],
                                 start=False, stop=True)
                nc.vector.tensor_scalar_add(out=shT_b[:], in0=sh_ps[:],
                                            scalar1=w3bc[:])
            else:
                nc.vector.memset(shT_b[:], 0.01)

            # --- per-block inv scale, all-DVE (Newton sqrt from above) ---
            sh2 = blkp.tile([P, P], f16, tag="sh2")
            nc.vector.tensor_tensor(out=sh2[:], in0=shT_b[:], in1=shT_b[:], op=MUL)
            ssp = ps2.tile([P, VT], f32, tag="sc", space="PSUM")
            nc.tensor.matmul(out=ssp[0:1, :P], lhsT=ones_c[:], rhs=sh2[:],
                             start=True, stop=True)
            ssr = blkp.tile([1, P], f16, tag="ssr")
            nc.vector.tensor_copy(out=ssr[:], in_=ssp[0:1, :P])
            stp = ps2.tile([P, VT], f32, tag="sc", space="PSUM")
            nc.tensor.matmul(out=stp[:, 0:1], lhsT=ssr[0:1, :],
                             rhs=ones_c[0:1, 0:1], start=True, stop=True)
            sqv = blkp.tile([P, 1], f32, tag="sqv")    # ss + 2e-4
            nc.vector.tensor_scalar(out=sqv[:], in0=stp[:, 0:1],
                                    scalar1=1.0, scalar2=2e-4,
                                    op0=MUL, op1=ADD)
            t = blkp.tile([P, 1], f32, tag="tnr")      # t0 = ss/4+1 >= sqrt
            nc.vector.tensor_scalar(out=t[:], in0=sqv[:],
                                    scalar1=0.25, scalar2=1.0,
                                    op0=MUL, op1=ADD)
            for _ in range(NR_ITERS):                  # t <- (t + ss/t)/2
                r = blkp.tile([P, 1], f32, tag="rnr")
                nc.vector.reciprocal(out=r[:], in_=t[:])
                m = blkp.tile([P, 1], f32, tag="mnr")
                nc.vector.tensor_tensor(out=m[:], in0=sqv[:], in1=r[:], op=MUL)
                t2 = blkp.tile([P, 1], f32, tag="tnr")
                nc.vector.tensor_tensor(out=t2[:], in0=t[:], in1=m[:], op=ADD)
                t = t2
                nc.vector.tensor_scalar_mul(out=t[:], in0=t[:], scalar1=0.5)
            wv = blkp.tile([P, 1], f32, tag="wnr")     # t * C/127
            nc.vector.tensor_scalar_mul(out=wv[:], in0=t[:], scalar1=k127)
            inv32 = gat.tile([P, 1], f32, tag=f"inv{blk}")
            nc.vector.reciprocal(out=inv32[:], in_=wv[:])
            with nc.allow_low_precision(reason="int8 scale grid"):
                nc.vector.tensor_copy(out=invOut[:, blk:blk + 1], in_=inv32[:])

            if p2:
                # ---- phase 2 for this block over resident item_weight ----
                for ch in range(NCH):
                    w_g = csize[ch]
                    nvt = -(-w_g // VT)
                    sc = p2o.tile([P, W], i8, tag="scsb")
                    for s in range(nvt):
                        w = min(VT, w_g - s * VT)
                        vsl = slice(s * VT, s * VT + w)
                        sc_ps = ps2.tile([P, VT], f32, tag="sc", space="PSUM")
                        nc.tensor.matmul(out=sc_ps[:, :w],
                                         lhsT=shT_b[:], rhs=itw[ch][:, vsl],
                                         start=True, stop=True)
                        if qidx * 5 % 12 < 5:
                            nc.vector.tensor_scalar_mul(
                                out=sc[:, vsl], in0=sc_ps[:, :w],
                                scalar1=inv32[:])
                        else:
                            nc.scalar.activation(
                                out=sc[:, vsl], in_=sc_ps[:, :w], func=Copy,
                                scale=inv32[:])
                        qidx += 1
                    nc.sync.dma_start(
                        out=y_ext[gsl, ch * W:ch * W + w_g], in_=sc[:, :w_g])
        nc.scalar.dma_start(out=inv_ext[:], in_=invOut[:])


def prep_inputs(session_embedding, item_weight, W1_w, W1_b, W2_w, W2_b,
                q_w, q_b, W3_w, W3_b, batch, num_graphs):
    """Host-side sharding/layout. Returns (in_maps, ntpb, ckey, V)."""
    x = np.asarray(session_embedding, dtype=np.float32)
    itw = np.asarray(item_weight, dtype=np.float32)
    batch = np.asarray(batch).astype(np.int64)
    B = int(num_graphs)
    N, d = x.shape
    Vv = itw.shape[0]
    assert d == D and B == NCORES * BC and Vv == V

    counts = np.bincount(batch, minlength=B)
    assert counts.min() >= 1, "every graph must be non-empty"
    starts = np.zeros(B + 1, np.int64)
    np.cumsum(counts, out=starts[1:])
    assert starts[-1] == N
    last_idx = starts[1:] - 1
    v_n = x[last_idx]

    blk_cnt = starts[P::P] - starts[:-P:P].reshape(-1)
    ntpb = int(-(-blk_cnt.max() // P))
    npb = ntpb * P

    itwh = itw.astype(np.float16)
    C = float(np.linalg.norm(itwh.astype(np.float64), axis=1).max())
    ckey = (C / 127.0) ** 2
    itwT = np.ascontiguousarray(itwh.T)

    w1t = np.ascontiguousarray(np.asarray(W1_w, np.float32).T).astype(np.float16)
    w2t = np.ascontiguousarray(np.asarray(W2_w, np.float32).T).astype(np.float16)
    W3 = np.asarray(W3_w, np.float32)
    w3at = np.ascontiguousarray(W3[:, :D].T).astype(np.float16)
    w3bt = np.ascontiguousarray(W3[:, D:].T).astype(np.float16)
    b12c = (np.asarray(W1_b, np.float32) + np.asarray(W2_b, np.float32)
            ).reshape(P, 1).copy()
    w3bc = np.asarray(W3_b, np.float32).reshape(P, 1).copy()
    qwt = np.ascontiguousarray(
        np.asarray(q_w, np.float32).reshape(1, D).T).astype(np.float16)
    qbc = np.full((P, 1), np.float32(np.asarray(q_b).reshape(())), np.float32)

    xh = x.astype(np.float16)
    in_maps = []
    for c in range(NCORES):
        xpad = np.zeros((NBLK, npb, D), np.float16)
        bl = np.zeros((NBLK, P, ntpb), np.float32)
        blr = np.zeros((NBLK, npb), np.float16)
        for b in range(NBLK):
            glo = c * BC + b * P
            s, e = int(starts[glo]), int(starts[glo + P])
            cnt = e - s
            assert cnt <= npb
            xpad[b, :cnt] = xh[s:e]
            locp = np.zeros(npb, np.float32)
            locp[:cnt] = (batch[s:e] - glo).astype(np.float32)
            bl[b] = locp.reshape(ntpb, P).T
            blr[b] = locp.astype(np.float16)
        xpk = np.ascontiguousarray(
            xpad.reshape(NBLK, ntpb, P, D).transpose(2, 0, 1, 3).reshape(P, NBLK * npb))
        xtp = np.ascontiguousarray(
            xpad.transpose(2, 0, 1).reshape(P, NBLK * npb))
        vntc = np.ascontiguousarray(v_n[c * BC:(c + 1) * BC].T).astype(np.float16)
        im = dict(
            xpk=xpk, xtp=xtp, blc=np.ascontiguousarray(bl),
            blr=np.ascontiguousarray(blr), vnt=vntc,
            w1t=w1t, w2t=w2t, w3at=w3at, w3bt=w3bt,
            b12c=b12c, w3bc=w3bc, qwt=qwt, qbc=qbc, itwt=itwT)
        in_maps.append(im)
    return in_maps, ntpb, ckey, V


_NC_CACHE = {}


def get_nc(ntpb, ckey, repeat=1, phase="both"):
    key = (ntpb, ckey, repeat, phase)
    if key not in _NC_CACHE:
        _NC_CACHE[key] = build_nc(ntpb, ckey, repeat, phase)
    return _NC_CACHE[key]


def unshard(results, Vv):
    """Assemble full fp32 [BTOT, V] output from per-core int8 row shards."""
    y = np.empty((BTOT, Vv), np.float32)
    for c in range(NCORES):
        inv = results[c]["invs"]                    # [128, 4] f16, graph b*128+p
        scale = (1.0 / inv.astype(np.float64).T.reshape(-1)).astype(np.float32)
        q = results[c]["y"].astype(np.float32)
        y[c * BC:(c + 1) * BC] = q * scale[:, None]
    return y


def kernel(**inputs):
    from concourse.bass_utils import run_bass_kernel_spmd

    in_maps, ntpb, ckey, Vv = prep_inputs(**inputs)
    nc = get_nc(ntpb, ckey)
    res = run_bass_kernel_spmd(nc, in_maps, core_ids=list(range(NCORES)))
    return unshard(res.results, Vv)
